# revision 1
# baseline (speedup 1.0000x reference)
"""Trainium2 Bass kernel: class-routed 2-layer MLP (MoE-style routing).

    out[b] = W2[y[b]] . tanh(W1[y[b]] @ Z[b] + b1[y[b]]) + b2[y[b]]

Sharding strategy (expert/class sharding, not batch sharding):
  - Classes present in y are assigned to the 8 cores by greedy
    load-balancing; samples are routed on the host to the core owning
    their class.
  - On each core the kernel iterates over "slots": one slot = one unique
    class plus up to S of its routed samples. Host packs, per slot,
    the class's transposed W1 row ([F,H] layout, f on partitions, fp16)
    so the device program is a fully static stream: one contiguous DMA
    per slot group carrying W1cT plus that slot's Z columns -> 16 matmuls
    -> tanh(+b1) -> small matmul with W2 -> output slot.
  - Deduplication: each class's W1 row is read from HBM once globally
    (vs once per sample for the naive gather), which is what the
    memory-bound roofline wants. Measured ~187 us on 8 cores (DMA
    bursts >400 GB/s/core, ~340 avg incl. head/tail), rel err ~1e-3.

All routing/gather/scatter is host-side numpy baked into the input
layout; the device NEFF is identical across cores (SPMD) and contains no
data-dependent control flow.
"""

import os
import numpy as np

N_CORES = 8
S = 8                       # sample capacity per class-slot
F = 512                     # feature dim (layer-1 contraction)
H = 512                     # hidden dim
FT = F // 128               # f-tiles
HT = H // 128               # h-tiles

# "float32" / "float16" / "bfloat16" for the streamed W1/W2/Z tensors.
# float16 keeps ~1e-3 relative accuracy while halving HBM traffic and
# running single-pass matmuls (fp32 matmuls are two-pass HI/LO on trn2).
W_DTYPE = os.environ.get("KERNEL_W_DTYPE", "float16")
DMA_GROUP = 2   # weight slots per dma_start (1 MB @ fp16 -> near-peak HBM bw)
OUT_GROUP = 32  # slots sharing one PSUM output bank before copy-out

# Set by kernel() after each run (ns, from neuron-profile; None w/o trace).
LAST_EXEC_TIME_NS = None
LAST_MEAN_EXEC_TIME_NS = None

_PROGRAM_CACHE = {}


def _route(y):
    """Group samples by class, balance classes across cores, build slots.

    Each present class contributes ceil(n_c / S) slots (one slot = one
    class + up to S samples). Classes are assigned to cores greedily
    (most slots first -> least-loaded core) so the per-core slot count —
    which sets the per-core HBM traffic — is near-uniform.

    Returns (slots_per_core, K): slots_per_core[m] is a list of
    (class_id, sample_index_array); K = max slot count over cores.
    """
    order = np.argsort(y, kind="stable")
    ys = y[order]
    uniq, starts, counts = np.unique(ys, return_index=True, return_counts=True)
    class_slots = []  # (n_slots, class_id, sample_idx_array)
    for u, s0, n in zip(uniq, starts, counts):
        class_slots.append((-(-int(n) // S), int(u), order[s0 : s0 + n]))
    class_slots.sort(key=lambda t: -t[0])
    loads = [0] * N_CORES
    slots_per_core = [[] for _ in range(N_CORES)]
    for nslots, cls, sidx in class_slots:
        m = loads.index(min(loads))
        loads[m] += nslots
        for j in range(0, len(sidx), S):
            slots_per_core[m].append((cls, sidx[j : j + S]))
    K = max(1, max(len(s) for s in slots_per_core))
    return slots_per_core, K


def _build_program(K, bias_zero):
    import concourse.mybir as mybir
    import concourse.tile as tile
    from concourse import bacc

    f32 = mybir.dt.float32
    wdt = {
        "float32": f32,
        "float16": mybir.dt.float16,
        "bfloat16": mybir.dt.bfloat16,
    }[W_DTYPE]
    G = DMA_GROUP
    assert K % G == 0

    C = FT * H + FT * S  # per-slot stream columns: W1cT then Z samples
    NG = K // G

    nc = bacc.Bacc("TRN2", debug=False)
    wz = nc.dram_tensor("wz", [NG, 128, G * C], wdt, kind="ExternalInput")
    b1s = None
    if not bias_zero:
        b1s = nc.dram_tensor("b1s", [128, K * HT], f32, kind="ExternalInput")
    l2dt = f32 if os.environ.get("KERNEL_L2_F32", "1") == "1" else wdt
    w2s = nc.dram_tensor("w2s", [128, K * HT], l2dt, kind="ExternalInput")
    out = nc.dram_tensor("out", [1, K * S], f32, kind="ExternalOutput")

    with tile.TileContext(nc) as tc:
        with (
            tc.tile_pool(name="consts", bufs=1) as cpool,
            tc.tile_pool(name="wp", bufs=12) as wpool,
            tc.tile_pool(name="thp", bufs=3) as thpool,
            tc.tile_pool(name="hps", bufs=2, space="PSUM") as hpool,
            tc.tile_pool(name="ops", bufs=2, space="PSUM") as opool,
        ):
            # Residents: biases and W2 (Z rides inside the weight stream).
            if not bias_zero:
                b1_sb = cpool.tile([128, K * HT], f32)
                nc.gpsimd.dma_start(b1_sb[:], b1s[:])
            w2_sb = cpool.tile([128, K * HT], l2dt)
            nc.gpsimd.dma_start(w2_sb[:], w2s[:])
            out_sb = cpool.tile([1, K * S], f32)

            o_ps = None
            for k in range(K):
                j, g = divmod(k, G)
                if g == 0:
                    # Alternate the two HWDGE rings (SP / ACT) so DMA
                    # setup+completion latencies overlap across rings; the
                    # first groups go on SP, whose preamble clears ~1.3 us
                    # before ACT's table load. One contiguous read each.
                    eng = nc.sync if (j < 3 or j % 2 == 0) else nc.scalar
                    w_sb = wpool.tile([128, G * C], wdt)
                    eng.dma_start(w_sb[:], wz[j])
                o = g * C

                h_ps = hpool.tile([128, HT, S], f32)
                for ht in range(HT):
                    for ft in range(FT):
                        nc.tensor.matmul(
                            h_ps[:, ht, :],
                            w_sb[:, o + ft * H + ht * 128 : o + ft * H + (ht + 1) * 128],
                            w_sb[:, o + FT * H + ft * S : o + FT * H + (ft + 1) * S],
                            start=(ft == 0),
                            stop=(ft == FT - 1),
                        )

                th_sb = thpool.tile([128, HT, S], l2dt)
                if bias_zero:
                    nc.scalar.activation(
                        th_sb[:], h_ps[:], mybir.ActivationFunctionType.Tanh
                    )
                else:
                    for ht in range(HT):
                        nc.scalar.activation(
                            th_sb[:, ht, :],
                            h_ps[:, ht, :],
                            mybir.ActivationFunctionType.Tanh,
                            bias=b1_sb[:, k * HT + ht : k * HT + ht + 1],
                        )

                if k % OUT_GROUP == 0:
                    o_ps = opool.tile([1, OUT_GROUP * S], f32)
                j = (k % OUT_GROUP) * S
                for ht in range(HT):
                    nc.tensor.matmul(
                        o_ps[:, j : j + S],
                        w2_sb[:, k * HT + ht : k * HT + ht + 1],
                        th_sb[:, ht, :],
                        start=(ht == 0),
                        stop=(ht == HT - 1),
                    )
                if k % OUT_GROUP == OUT_GROUP - 1 or k == K - 1:
                    k0 = (k // OUT_GROUP) * OUT_GROUP
                    n = (k - k0 + 1) * S
                    nc.vector.tensor_copy(
                        out_sb[:, k0 * S : k0 * S + n], o_ps[:, :n]
                    )
                    nc.sync.dma_start(
                        out[:, k0 * S : k0 * S + n],
                        out_sb[:, k0 * S : k0 * S + n],
                    )

    nc.compile()
    return nc


def _install_profile_hook():
    """Register the axon NTFF profiling hook if the image lacks
    antenv.axon_hooks (degrades to no trace if anything is missing)."""
    import sys
    import types

    try:
        from antenv.axon_hooks import get_axon_ntff_profile_hook  # noqa: F401

        return
    except ImportError:
        pass
    try:
        import antenv
        from trn_agent_boot.trn_boot import _ntff_profile_via_ctypes

        so = "/opt/axon/libaxon_pjrt.so"
        if not os.path.exists(so):
            return
        mod = types.ModuleType("antenv.axon_hooks")
        holder = [None]
        mod.set_axon_ntff_profile_hook = lambda h: holder.__setitem__(0, h)
        mod.get_axon_ntff_profile_hook = lambda: holder[0]
        sys.modules["antenv.axon_hooks"] = mod
        antenv.axon_hooks = mod
        mod.set_axon_ntff_profile_hook(_ntff_profile_via_ctypes(so))
    except Exception:
        pass


def kernel(Z, y, W1, b1, W2, b2):
    global LAST_EXEC_TIME_NS, LAST_MEAN_EXEC_TIME_NS
    import sys

    if "jax" not in sys.modules:
        os.environ.setdefault("JAX_PLATFORMS", "axon")
    from concourse.bass_utils import run_bass_kernel_spmd

    Z = np.asarray(Z, dtype=np.float32)
    y = np.asarray(y).astype(np.int64)
    W1 = np.asarray(W1, dtype=np.float32)
    b1 = np.asarray(b1, dtype=np.float32)
    W2 = np.asarray(W2, dtype=np.float32)
    b2 = np.asarray(b2, dtype=np.float32)
    B = Z.shape[0]
    assert Z.shape == (B, F) and W1.shape[1:] == (H, F)

    if W_DTYPE == "float32":
        wnp = np.float32
    elif W_DTYPE == "float16":
        wnp = np.float16
    else:
        import ml_dtypes

        wnp = ml_dtypes.bfloat16

    slots_per_core, K = _route(y)
    K = ((K + DMA_GROUP - 1) // DMA_GROUP) * DMA_GROUP
    bias_zero = not np.any(b1)
    key = (K, W_DTYPE, bias_zero, os.environ.get("KERNEL_L2_F32", "1"))
    if key not in _PROGRAM_CACHE:
        _PROGRAM_CACHE[key] = _build_program(K, bias_zero)
    nc = _PROGRAM_CACHE[key]

    Zt = np.ascontiguousarray(Z.T)  # [F, B]
    G = DMA_GROUP
    C = FT * H + FT * S
    NG = K // G
    in_maps = []
    for m in range(N_CORES):
        slots = slots_per_core[m]
        cls_list = np.array(
            [c for c, _ in slots] + [0] * (K - len(slots)), dtype=np.int64
        )
        # Combined stream: per slot, W1cT columns then Z sample columns.
        # wz[j, p, g*C + ft*H + h]      = W1[cls_{jG+g}, h, ft*128 + p]
        # wz[j, p, g*C + FT*H + ft*S+s] = Z[sample_s_of_slot, ft*128 + p]
        wzm = np.empty((NG, 128, G * C), wnp)
        wzv = wzm.reshape(NG, 128, G, C)
        np.copyto(
            wzv[:, :, :, : FT * H].reshape(NG, 128, G, FT, H),
            W1[cls_list].reshape(NG, G, H, FT, 128).transpose(0, 4, 1, 3, 2),
            casting="same_kind",
        )
        zpart = wzv[:, :, :, FT * H :].reshape(NG, 128, G, FT, S)
        zpart[...] = 0
        for k, (_, sidx) in enumerate(slots):
            j, g = divmod(k, G)
            zpart[j, :, g, :, : len(sidx)] = (
                Zt[:, sidx].reshape(FT, 128, len(sidx)).transpose(1, 0, 2)
            )
        # b1s[p, k*HT + ht] = b1[cls_k, ht*128 + p]; same layout for w2s
        l2np = np.float32 if os.environ.get("KERNEL_L2_F32", "1") == "1" else wnp
        w2sm = (
            W2[cls_list]
            .reshape(K, HT, 128)
            .transpose(2, 0, 1)
            .astype(l2np)
            .reshape(128, K * HT)
        )
        im = {"wz": wzm, "w2s": w2sm}
        if not bias_zero:
            im["b1s"] = np.ascontiguousarray(
                b1[cls_list].reshape(K, HT, 128).transpose(2, 0, 1)
            ).reshape(128, K * HT)
        in_maps.append(im)

    trace = os.environ.get("KERNEL_TRACE", "0") == "1"
    if trace:
        _install_profile_hook()
    res = run_bass_kernel_spmd(
        nc, in_maps, core_ids=list(range(N_CORES)), trace=trace
    )
    LAST_EXEC_TIME_NS = res.exec_time_ns
    LAST_MEAN_EXEC_TIME_NS = res.mean_exec_time_ns

    out = np.empty(B, dtype=np.float32)
    for m in range(N_CORES):
        o = np.asarray(res.results[m]["out"]).reshape(K * S)
        for k, (_, sidx) in enumerate(slots_per_core[m]):
            out[sidx] = o[k * S : k * S + len(sidx)]
    out += b2[y]
    return out

